# revision 4
# baseline (speedup 1.0000x reference)
"""Trainium2 Bass kernel for nn_ConvBlock: 3x3 conv (stride 1, pad 1) with
MRR phase-encoded weights + BatchNorm(eval) + ReLU6.

Sharding: data-parallel over batch across 8 cores (8 images each); the small
phase/BN params are replicated. Per core:
 - Weights: DMA phase blocks into [oc, ic*9] layout, apply the MRR
   phase->transmission transform elementwise (DVE+ACT), PE-transpose
   per-tap slices into [ic, oc] matmul operands.
 - Conv: for each image / oc-group(128) / 8-row strip: accumulating
   matmuls (K=128, N=448) over a zero-padded [128, 58, 58] input.
 - Precision mode "bf16x3" (default): Dekker-style split w=w1+w2,
   x=x1+x2 (bf16 halves); accumulate w1x1 + w1x2 + w2x1 in fp32 PSUM.
   bf16 operands stream at 1 cycle/row so this is ~2.6x cheaper than
   native fp32 matmuls (4 cycles/row) with ~1e-3 of the output scale
   worst-case error. Modes "f32r"/"f32" kept for comparison.
 - Epilogue: ACT affine(BN)+ReLU from PSUM, DVE min(.,6), DMA out.
"""
import math

import numpy as np

import concourse.bass as bass
import concourse.mybir as mybir
import concourse.tile as tile
from concourse import bacc
from concourse.bass_utils import run_bass_kernel_spmd
from concourse.masks import make_identity

F32 = mybir.dt.float32
F32R = mybir.dt.float32r
BF16 = mybir.dt.bfloat16
AF = mybir.ActivationFunctionType
ALU = mybir.AluOpType

MRR_A = 0.987
MRR_R = 0.99
BN_EPS = 1e-5
PI = math.pi
MAGIC = 12582912.0  # 1.5 * 2**23: fp32 round-to-nearest-int magic constant

N_CORES = 8
IMG_PER_CORE = 8
C_IN = 128
C_OUT = 256
H = W = 56
HP = WP = 58  # padded
N_STRIPS = 7
ROWS_PER_STRIP = 8
NFREE = ROWS_PER_STRIP * W  # 448

_NC_CACHE = {}


def _weight_transform(nc, wpool, wpsum_pool, phase, mode):
    """phase -> MRR transmission -> per-tap [ic, oc] lhsT tiles.

    Returns (w1, w2): bf16x3 mode gives bf16 hi/lo pair; other modes give
    (wt, None) in the matmul dtype.
    """
    ar = MRR_A * MRR_R
    kconst = (1.0 - MRR_A * MRR_A) * (1.0 - MRR_R * MRR_R)

    ident = wpool.tile([128, 128], F32)
    make_identity(nc, ident)
    if mode == "bf16x3":
        w1 = wpool.tile([128, 9, 2, 128], BF16)
        w2 = wpool.tile([128, 9, 2, 128], BF16)
    else:
        dt = F32R if mode == "f32r" else F32
        w1 = wpool.tile([128, 9, 2, 128], dt)
        w2 = None

    for g in range(2):
        ph = wpool.tile([128, 1152], F32, tag="ph")
        ph_v = ph[:, :].rearrange("(a i) f -> a i f", i=8)
        for i in range(8):
            nc.sync.dma_start(out=ph_v[:, i, :],
                              in_=phase[16 * g:16 * (g + 1), :, i, :])
        # t = phi + pi/2 wrapped to [-pi, pi] (exact fp32 rounding trick)
        q = wpool.tile([128, 1152], F32, tag="q")
        nc.vector.tensor_scalar_add(ph, ph, PI / 2)
        nc.vector.tensor_scalar(q, ph, 1.0 / (2 * PI), MAGIC, ALU.mult, ALU.add)
        nc.vector.tensor_scalar_sub(q, q, MAGIC)
        nc.vector.scalar_tensor_tensor(ph, q, -2.0 * PI, ph, ALU.mult, ALU.add)
        # cos(phi) = sin(t); den = 1+(ar)^2-2ar*c; tr = 1 - K/den
        nc.scalar.activation(ph, ph, AF.Sin)
        nc.scalar.activation(ph, ph, AF.Copy, bias=1.0 + ar * ar, scale=-2.0 * ar)
        nc.vector.reciprocal(ph, ph)
        nc.scalar.activation(ph, ph, AF.Copy, bias=1.0, scale=-kconst)
        # transpose each tap slice [oc, ic] -> [ic, oc]
        ph_k = ph[:, :].rearrange("p (ic k) -> p ic k", k=9)
        for k in range(9):
            tr_ps = wpsum_pool.tile([128, 128], F32)
            nc.tensor.transpose(tr_ps[:, :], ph_k[:, :, k], ident[:, :])
            if mode == "bf16x3":
                # w1 = bf16(w); w2 = bf16(w - f32(w1))
                nc.vector.tensor_copy(w1[:, k, g, :], tr_ps[:, :])
                res = wpool.tile([128, 128], F32, tag="wres")
                nc.vector.scalar_tensor_tensor(res, w1[:, k, g, :], -1.0,
                                               tr_ps[:, :], ALU.mult, ALU.add)
                nc.vector.tensor_copy(w2[:, k, g, :], res)
            else:
                nc.vector.tensor_copy(w1[:, k, g, :], tr_ps[:, :])
    return w1, w2


def build(mode="bf16x3"):
    assert mode in ("bf16x3", "f32r", "f32")
    nc = bacc.Bacc(None, target_bir_lowering=False)
    x = nc.dram_tensor("x", (IMG_PER_CORE, C_IN, H, W), F32, kind="ExternalInput")
    phase = nc.dram_tensor("phase", (32, 144, 8, 8), F32, kind="ExternalInput")
    gamma = nc.dram_tensor("gamma", (C_OUT,), F32, kind="ExternalInput")
    beta = nc.dram_tensor("beta", (C_OUT,), F32, kind="ExternalInput")
    rmean = nc.dram_tensor("running_mean", (C_OUT,), F32, kind="ExternalInput")
    rvar = nc.dram_tensor("running_var", (C_OUT,), F32, kind="ExternalInput")
    y = nc.dram_tensor("y", (IMG_PER_CORE, C_OUT, H, W), F32, kind="ExternalOutput")

    with tile.TileContext(nc) as tc:
        with tc.tile_pool(name="wpool", bufs=1) as wpool, \
             tc.tile_pool(name="xpool", bufs=2) as xpool, \
             tc.tile_pool(name="ypool", bufs=3) as ypool, \
             tc.tile_pool(name="psum", bufs=6, space="PSUM") as psum_pool, \
             tc.tile_pool(name="wpsum", bufs=2, space="PSUM") as wpsum_pool:

            w1, w2 = _weight_transform(nc, wpool, wpsum_pool, phase, mode)

            # ---------------- BN constants -> per-partition scale/bias
            eps_t = wpool.tile([128, 1], F32)
            nc.vector.memset(eps_t, BN_EPS)
            gm = wpool.tile([128, 2], F32)
            bt = wpool.tile([128, 2], F32)
            mn = wpool.tile([128, 2], F32)
            vr = wpool.tile([128, 2], F32)
            nc.sync.dma_start(out=gm, in_=gamma[:].rearrange("(g p) -> p g", p=128))
            nc.sync.dma_start(out=bt, in_=beta[:].rearrange("(g p) -> p g", p=128))
            nc.sync.dma_start(out=mn, in_=rmean[:].rearrange("(g p) -> p g", p=128))
            nc.sync.dma_start(out=vr, in_=rvar[:].rearrange("(g p) -> p g", p=128))
            inv = wpool.tile([128, 2], F32)
            nc.scalar.activation(inv, vr, AF.Sqrt, bias=eps_t[:, 0:1])
            nc.vector.reciprocal(inv, inv)
            nc.vector.tensor_mul(inv, inv, gm)
            bias_eff = wpool.tile([128, 2], F32)
            nc.vector.tensor_mul(bias_eff, mn, inv)
            nc.vector.tensor_sub(bias_eff, bt, bias_eff)

            zrow = wpool.tile([128, HP], F32)
            nc.vector.memset(zrow, 0.0)

            # ---------------- conv main loop
            for n in range(IMG_PER_CORE):
                if mode == "bf16x3":
                    xs = xpool.tile([128, HP, WP], F32, tag="xs")
                    nc.vector.memset(xs[:, 0, :], 0.0)
                    nc.vector.memset(xs[:, HP - 1, :], 0.0)
                    nc.vector.memset(xs[:, :, 0], 0.0)
                    nc.vector.memset(xs[:, :, WP - 1], 0.0)
                    nc.sync.dma_start(out=xs[:, 1:57, 1:57], in_=x[n])
                    xp1 = xpool.tile([128, HP, WP], BF16, tag="xp1")
                    xp2 = xpool.tile([128, HP, WP], BF16, tag="xp2")
                    nc.vector.tensor_copy(xp1, xs)
                    xr = xpool.tile([128, HP, WP], F32, tag="xr")
                    nc.vector.scalar_tensor_tensor(xr, xp1, -1.0, xs,
                                                   ALU.mult, ALU.add)
                    nc.vector.tensor_copy(xp2, xr)
                    rhs_tiles = (xp1, xp2)
                else:
                    dt = F32R if mode == "f32r" else F32
                    xpad = xpool.tile([128, HP, WP], dt, tag="xs")
                    nc.vector.tensor_copy(xpad[:, 0, :], zrow[:, :])
                    nc.vector.tensor_copy(xpad[:, HP - 1, :], zrow[:, :])
                    nc.vector.tensor_copy(xpad[:, :, 0], zrow[:, :])
                    nc.vector.tensor_copy(xpad[:, :, WP - 1], zrow[:, :])
                    nc.sync.dma_start(out=xpad[:, 1:57, 1:57], in_=x[n].bitcast(dt))
                    rhs_tiles = (xpad,)

                for g in range(2):
                    ytile = ypool.tile([128, H, W], F32, tag="ytile")
                    for s in range(N_STRIPS):
                        ps = psum_pool.tile([128, ROWS_PER_STRIP, W], F32)
                        r0 = s * ROWS_PER_STRIP
                        if mode == "bf16x3":
                            terms = [(w1, rhs_tiles[0]), (w1, rhs_tiles[1]),
                                     (w2, rhs_tiles[0])]
                        else:
                            terms = [(w1, rhs_tiles[0])]
                        n_mm = 9 * len(terms)
                        i_mm = 0
                        for k in range(9):
                            dh, dw = k // 3 - 1, k % 3 - 1
                            for wt_t, xt in terms:
                                rhs = xt[:, r0 + dh + 1:r0 + dh + 9,
                                         dw + 1:dw + 57]
                                nc.tensor.matmul(ps[:, :, :], wt_t[:, k, g, :],
                                                 rhs, start=(i_mm == 0),
                                                 stop=(i_mm == n_mm - 1))
                                i_mm += 1
                        nc.scalar.activation(ytile[:, r0:r0 + ROWS_PER_STRIP, :],
                                             ps[:, :, :], AF.Relu,
                                             bias=bias_eff[:, g:g + 1],
                                             scale=inv[:, g:g + 1])
                    nc.vector.tensor_scalar_min(ytile, ytile, 6.0)
                    nc.sync.dma_start(out=y[n, 128 * g:128 * (g + 1), :, :],
                                      in_=ytile[:, :, :])
    nc.compile()
    return nc


def _run(inputs, trace=False, mode="bf16x3", trace_cores=None):
    if mode not in _NC_CACHE:
        _NC_CACHE[mode] = build(mode)
    nc = _NC_CACHE[mode]
    x = np.ascontiguousarray(inputs["x"], dtype=np.float32)
    common = {
        "phase": np.ascontiguousarray(inputs["phase"], dtype=np.float32),
        "gamma": np.ascontiguousarray(inputs["gamma"], dtype=np.float32),
        "beta": np.ascontiguousarray(inputs["beta"], dtype=np.float32),
        "running_mean": np.ascontiguousarray(inputs["running_mean"],
                                             dtype=np.float32),
        "running_var": np.ascontiguousarray(inputs["running_var"],
                                            dtype=np.float32),
    }
    in_maps = [
        {"x": np.ascontiguousarray(x[c * IMG_PER_CORE:(c + 1) * IMG_PER_CORE]),
         **common}
        for c in range(N_CORES)
    ]
    res = run_bass_kernel_spmd(nc, in_maps, core_ids=list(range(N_CORES)),
                               trace=trace, trace_cores=trace_cores)
    out = np.concatenate([r["y"] for r in res.results], axis=0)
    return out, res


def kernel(**inputs) -> np.ndarray:
    out, _ = _run(inputs)
    return out


# revision 5
# speedup vs baseline: 1.0271x; 1.0271x over previous
"""Trainium2 Bass kernel for nn_ConvBlock: 3x3 conv (stride 1, pad 1) with
MRR phase-encoded weights + BatchNorm(eval) + ReLU6.

Sharding: data-parallel over batch across 8 cores (8 images each); the small
phase/BN params are replicated. Per core:
 - Weights: DMA phase blocks into [oc, ic*9] layout, apply the MRR
   phase->transmission transform elementwise (DVE+ACT), PE-transpose
   per-tap slices into [ic, oc] matmul operands.
 - Conv: for each image / oc-group(128) / 8-row strip: accumulating
   matmuls (K=128, N=448) over a zero-padded [128, 58, 58] input.
 - Precision mode "bf16x3" (default): Dekker-style split w=w1+w2,
   x=x1+x2 (bf16 halves); accumulate w1x1 + w1x2 + w2x1 in fp32 PSUM.
   bf16 operands stream at 1 cycle/row so this is ~2.6x cheaper than
   native fp32 matmuls (4 cycles/row) with ~5e-3-of-scale worst-case
   error. Modes "f32r"/"f32" kept for comparison.
 - Epilogue: ACT affine(BN)+ReLU from PSUM, DVE min(.,6), per-strip DMA out.

Program order interleaves group-0 weight prep, image-0 conv, and group-1
weight prep so the PE never waits for the full weight pipeline.
"""
import math

import numpy as np

import concourse.bass as bass
import concourse.mybir as mybir
import concourse.tile as tile
from concourse import bacc
from concourse.bass_utils import run_bass_kernel_spmd
from concourse.masks import make_identity

F32 = mybir.dt.float32
F32R = mybir.dt.float32r
BF16 = mybir.dt.bfloat16
AF = mybir.ActivationFunctionType
ALU = mybir.AluOpType

MRR_A = 0.987
MRR_R = 0.99
BN_EPS = 1e-5
PI = math.pi
MAGIC = 12582912.0  # 1.5 * 2**23: fp32 round-to-nearest-int magic constant

N_CORES = 8
IMG_PER_CORE = 8
C_IN = 128
C_OUT = 256
H = W = 56
HP = WP = 58  # padded
N_STRIPS = 7
ROWS_PER_STRIP = 8
NFREE = ROWS_PER_STRIP * W  # 448

_NC_CACHE = {}


def _mrr_chain(nc, wpool, phase, g):
    """DMA phase block g and apply the MRR transform -> tr [oc=128, ic*9]."""
    ar = MRR_A * MRR_R
    kconst = (1.0 - MRR_A * MRR_A) * (1.0 - MRR_R * MRR_R)
    ph = wpool.tile([128, 1152], F32, tag=f"ph{g}")
    ph_v = ph[:, :].rearrange("(a i) f -> a i f", i=8)
    for i in range(8):
        nc.sync.dma_start(out=ph_v[:, i, :],
                          in_=phase[16 * g:16 * (g + 1), :, i, :])
    # t = phi + pi/2 wrapped to [-pi, pi] (exact fp32 rounding trick)
    q = wpool.tile([128, 1152], F32, tag=f"q{g}")
    nc.vector.tensor_scalar_add(ph, ph, PI / 2)
    nc.vector.tensor_scalar(q, ph, 1.0 / (2 * PI), MAGIC, ALU.mult, ALU.add)
    nc.vector.tensor_scalar_sub(q, q, MAGIC)
    nc.vector.scalar_tensor_tensor(ph, q, -2.0 * PI, ph, ALU.mult, ALU.add)
    # cos(phi) = sin(t); den = 1+(ar)^2-2ar*c; tr = 1 - K/den
    nc.scalar.activation(ph, ph, AF.Sin)
    nc.scalar.activation(ph, ph, AF.Copy, bias=1.0 + ar * ar, scale=-2.0 * ar)
    nc.vector.reciprocal(ph, ph)
    nc.scalar.activation(ph, ph, AF.Copy, bias=1.0, scale=-kconst)
    return ph


def _weight_taps(nc, wpool, wpsum_pool, ph, ident, w1, w2, g, mode):
    """Transpose each tap slice [oc, ic] -> [ic, oc] and split hi/lo."""
    ph_k = ph[:, :].rearrange("p (ic k) -> p ic k", k=9)
    for k in range(9):
        tr_ps = wpsum_pool.tile([128, 128], F32, tag="tr")
        nc.tensor.transpose(tr_ps[:, :], ph_k[:, :, k], ident[:, :])
        if mode == "bf16x3":
            # stage to SBUF on ACT (keeps DVE free), then split hi/lo
            stg = wpool.tile([128, 128], F32, tag="wstg")
            nc.scalar.copy(stg[:, :], tr_ps[:, :])
            nc.vector.tensor_copy(w1[:, k, g, :], stg[:, :])
            res = wpool.tile([128, 128], F32, tag="wres")
            nc.vector.scalar_tensor_tensor(res, w1[:, k, g, :], -1.0,
                                           stg[:, :], ALU.mult, ALU.add)
            nc.vector.tensor_copy(w2[:, k, g, :], res)
        else:
            nc.vector.tensor_copy(w1[:, k, g, :], tr_ps[:, :])


def build(mode="bf16x3"):
    assert mode in ("bf16x3", "f32r", "f32")
    nc = bacc.Bacc(None, target_bir_lowering=False)
    x = nc.dram_tensor("x", (IMG_PER_CORE, C_IN, H, W), F32, kind="ExternalInput")
    phase = nc.dram_tensor("phase", (32, 144, 8, 8), F32, kind="ExternalInput")
    gamma = nc.dram_tensor("gamma", (C_OUT,), F32, kind="ExternalInput")
    beta = nc.dram_tensor("beta", (C_OUT,), F32, kind="ExternalInput")
    rmean = nc.dram_tensor("running_mean", (C_OUT,), F32, kind="ExternalInput")
    rvar = nc.dram_tensor("running_var", (C_OUT,), F32, kind="ExternalInput")
    y = nc.dram_tensor("y", (IMG_PER_CORE, C_OUT, H, W), F32, kind="ExternalOutput")

    with tile.TileContext(nc) as tc:
        with tc.tile_pool(name="wpool", bufs=1) as wpool, \
             tc.tile_pool(name="xpool", bufs=2) as xpool, \
             tc.tile_pool(name="ypool", bufs=3) as ypool, \
             tc.tile_pool(name="psum", bufs=6, space="PSUM") as psum_pool, \
             tc.tile_pool(name="wpsum", bufs=2, space="PSUM") as wpsum_pool:

            ident = wpool.tile([128, 128], F32)
            make_identity(nc, ident)
            if mode == "bf16x3":
                w1 = wpool.tile([128, 9, 2, 128], BF16)
                w2 = wpool.tile([128, 9, 2, 128], BF16)
            else:
                w1 = wpool.tile([128, 9, 2, 128], F32R if mode == "f32r" else F32)
                w2 = None

            # ---------------- BN constants -> per-partition scale/bias
            eps_t = wpool.tile([128, 1], F32)
            nc.vector.memset(eps_t, BN_EPS)
            gm = wpool.tile([128, 2], F32)
            bt = wpool.tile([128, 2], F32)
            mn = wpool.tile([128, 2], F32)
            vr = wpool.tile([128, 2], F32)
            nc.sync.dma_start(out=gm, in_=gamma[:].rearrange("(g p) -> p g", p=128))
            nc.sync.dma_start(out=bt, in_=beta[:].rearrange("(g p) -> p g", p=128))
            nc.sync.dma_start(out=mn, in_=rmean[:].rearrange("(g p) -> p g", p=128))
            nc.sync.dma_start(out=vr, in_=rvar[:].rearrange("(g p) -> p g", p=128))
            inv = wpool.tile([128, 2], F32)
            nc.scalar.activation(inv, vr, AF.Sqrt, bias=eps_t[:, 0:1])
            nc.vector.reciprocal(inv, inv)
            nc.vector.tensor_mul(inv, inv, gm)
            bias_eff = wpool.tile([128, 2], F32)
            nc.vector.tensor_mul(bias_eff, mn, inv)
            nc.vector.tensor_sub(bias_eff, bt, bias_eff)

            zrow = wpool.tile([128, HP], F32)
            nc.vector.memset(zrow, 0.0)

            def load_image(n):
                """DMA + pad + (bf16x3) hi/lo split for image n."""
                if mode == "bf16x3":
                    xs = xpool.tile([128, HP, WP], F32, tag="xs")
                    nc.vector.memset(xs[:, 0, :], 0.0)
                    nc.vector.memset(xs[:, HP - 1, :], 0.0)
                    nc.vector.memset(xs[:, :, 0], 0.0)
                    nc.vector.memset(xs[:, :, WP - 1], 0.0)
                    # split the load across two DMAs (separate queues)
                    nc.sync.dma_start(out=xs[:, 1:29, 1:57], in_=x[n, :, 0:28, :])
                    nc.sync.dma_start(out=xs[:, 29:57, 1:57], in_=x[n, :, 28:56, :])
                    xp1 = xpool.tile([128, HP, WP], BF16, tag="xp1")
                    xp2 = xpool.tile([128, HP, WP], BF16, tag="xp2")
                    nc.vector.tensor_copy(xp1, xs)
                    xr = xpool.tile([128, HP, WP], F32, tag="xr")
                    nc.vector.scalar_tensor_tensor(xr, xp1, -1.0, xs,
                                                   ALU.mult, ALU.add)
                    nc.vector.tensor_copy(xp2, xr)
                    return (xp1, xp2)
                dt = F32R if mode == "f32r" else F32
                xpad = xpool.tile([128, HP, WP], dt, tag="xs")
                nc.vector.tensor_copy(xpad[:, 0, :], zrow[:, :])
                nc.vector.tensor_copy(xpad[:, HP - 1, :], zrow[:, :])
                nc.vector.tensor_copy(xpad[:, :, 0], zrow[:, :])
                nc.vector.tensor_copy(xpad[:, :, WP - 1], zrow[:, :])
                nc.sync.dma_start(out=xpad[:, 1:29, 1:57],
                                  in_=x[n, :, 0:28, :].bitcast(dt))
                nc.sync.dma_start(out=xpad[:, 29:57, 1:57],
                                  in_=x[n, :, 28:56, :].bitcast(dt))
                return (xpad,)

            def conv_group(rhs_tiles, n, g):
                ytile = ypool.tile([128, H, W], F32, tag="ytile")
                for s in range(N_STRIPS):
                    ps = psum_pool.tile([128, ROWS_PER_STRIP, W], F32)
                    r0 = s * ROWS_PER_STRIP
                    if mode == "bf16x3":
                        terms = [(w1, rhs_tiles[0]), (w1, rhs_tiles[1]),
                                 (w2, rhs_tiles[0])]
                    else:
                        terms = [(w1, rhs_tiles[0])]
                    n_mm = 9 * len(terms)
                    i_mm = 0
                    for k in range(9):
                        dh, dw = k // 3 - 1, k % 3 - 1
                        for wt_t, xt in terms:
                            rhs = xt[:, r0 + dh + 1:r0 + dh + 9, dw + 1:dw + 57]
                            nc.tensor.matmul(ps[:, :, :], wt_t[:, k, g, :],
                                             rhs, start=(i_mm == 0),
                                             stop=(i_mm == n_mm - 1))
                            i_mm += 1
                    nc.scalar.activation(ytile[:, r0:r0 + ROWS_PER_STRIP, :],
                                         ps[:, :, :], AF.Relu,
                                         bias=bias_eff[:, g:g + 1],
                                         scale=inv[:, g:g + 1])
                    nc.vector.tensor_scalar_min(
                        ytile[:, r0:r0 + ROWS_PER_STRIP, :],
                        ytile[:, r0:r0 + ROWS_PER_STRIP, :], 6.0)
                    nc.scalar.dma_start(
                        out=y[n, 128 * g:128 * (g + 1),
                              r0:r0 + ROWS_PER_STRIP, :],
                        in_=ytile[:, r0:r0 + ROWS_PER_STRIP, :])

            # ---------------- interleaved schedule
            # g0 weights -> image0 g0 conv (covers g1 weight prep latency)
            ph0 = _mrr_chain(nc, wpool, phase, 0)
            _weight_taps(nc, wpool, wpsum_pool, ph0, ident, w1, w2, 0, mode)
            rhs0 = load_image(0)
            conv_group(rhs0, 0, 0)
            ph1 = _mrr_chain(nc, wpool, phase, 1)
            _weight_taps(nc, wpool, wpsum_pool, ph1, ident, w1, w2, 1, mode)
            conv_group(rhs0, 0, 1)
            for n in range(1, IMG_PER_CORE):
                rhs = load_image(n)
                for g in range(2):
                    conv_group(rhs, n, g)
    nc.compile()
    return nc


def _run(inputs, trace=False, mode="bf16x3", trace_cores=None):
    if mode not in _NC_CACHE:
        _NC_CACHE[mode] = build(mode)
    nc = _NC_CACHE[mode]
    x = np.ascontiguousarray(inputs["x"], dtype=np.float32)
    common = {
        "phase": np.ascontiguousarray(inputs["phase"], dtype=np.float32),
        "gamma": np.ascontiguousarray(inputs["gamma"], dtype=np.float32),
        "beta": np.ascontiguousarray(inputs["beta"], dtype=np.float32),
        "running_mean": np.ascontiguousarray(inputs["running_mean"],
                                             dtype=np.float32),
        "running_var": np.ascontiguousarray(inputs["running_var"],
                                            dtype=np.float32),
    }
    in_maps = [
        {"x": np.ascontiguousarray(x[c * IMG_PER_CORE:(c + 1) * IMG_PER_CORE]),
         **common}
        for c in range(N_CORES)
    ]
    res = run_bass_kernel_spmd(nc, in_maps, core_ids=list(range(N_CORES)),
                               trace=trace, trace_cores=trace_cores)
    out = np.concatenate([r["y"] for r in res.results], axis=0)
    return out, res


def kernel(**inputs) -> np.ndarray:
    out, _ = _run(inputs)
    return out
